# revision 42
# baseline (speedup 1.0000x reference)
"""Trainium2 Bass kernel for a single transformer decoder layer.

Reference semantics (B=64, T=200, E=512, H=8, D=64):
  x += SelfAttn(LN1(x))   (q,k row-masked by pred_mask, causal)
  x += CrossAttn(LN2(x))  (k from raw memory row-masked by src_mask,
                           v from LN2(x) (!), causal)
  x += FFN(LN3(x))        (512 -> 2048 -> relu -> 512)

Sharding: data-parallel over batch, 8 elems per NeuronCore, no collectives.

Layout strategy (per core, batch elems processed in PAIRS):
  - residual stream x kept NATURAL [t_chunk<=128, 512] in fp32
  - LN via bn_stats/bn_aggr (DVE); rstd = exp(-0.5*ln(var+eps)) on ACT;
    the (x*rstd + nb) apply runs on GPSIMD (tensor_scalar) so the ACT
    queue never blocks the PE's transposes; LN gamma folded into weights
  - LN for stage N+1 is EMITTED a full pipeline stage early, so its
    engine-queue work drains long before the PE needs the h tiles
    (kills the PE idle gaps that made the HAM clock-gate oscillate)
  - activations transposed to [E, 2*T] pair tiles via PE is_transpose
    matmuls; memory comes HOST-TRANSPOSED so CA needs no PE transposes
  - every contraction >=256 runs fp8 DoubleRow: q/k/w1 (stationary
    host-packed), v / oproj / FFN-w2 (stationary = fp8 activation pair
    tiles, j-stride a multiple of 16B)
  - scores computed TRANSPOSED per head per elem, 2 heads per PSUM bank;
    odd heads read q/k rows 64:128 in place (row_grp 64) -- no SBUF
    partition-shift DMAs; exp on ACT; causal mask via affine_select
  - softmax: denominators stacked into an [8,200] PSUM tile per elem via
    one-hot stationaries; NORMALIZATION APPLIED TO THE AV OUTPUT (128x200
    per unit instead of the ~2x bigger exp tiles): a pair-one-hot matmul
    broadcasts 1/d to the two 64-row head blocks and the AV drain is one
    DVE tensor_mul (psum x psum -> fp8 o8); the per-channel v bias is
    folded host-side through Wo into the output-projection bias
  - output-projection / FFN biases enter PSUM via rank-1 (K=1) matmuls
    placed FIRST in the accumulation group; FFN b1 rides the relu
    activation bias (per-partition in the transposed layout)
"""

import numpy as np
import ml_dtypes
from contextlib import ExitStack

import concourse.bass as bass
import concourse.bacc as bacc
import concourse.tile as tile
from concourse import mybir
from concourse.bass_utils import run_bass_kernel_spmd

B, T, E, H, Dh, F = 64, 200, 512, 8, 64, 2048
NCORES = 8
SCALE = float(E) ** -0.5
F32 = mybir.dt.float32
BF16 = mybir.dt.bfloat16
FP8 = mybir.dt.float8e4
AL = mybir.AluOpType
AF = mybir.ActivationFunctionType
DR = mybir.MatmulPerfMode.DoubleRow
TCH = [(0, 128), (128, 72)]  # token chunks (t0, tc)
ECH = E // 128  # 4
FCH = F // 128  # 16
NPBF16 = ml_dtypes.bfloat16
NPFP8 = ml_dtypes.float8_e4m3  # IEEE-style e4m3 (max 240) == TRN FP8_EXP4
S8 = 64.0  # fp8 weight pre-scale so N(0,0.02) weights stay in normal range

_programs = {}


def _layernorm(nc, pools, x_c, tc, eps, tag):
    """x_c: [tc,512] f32 natural -> (x-mu)*rsqrt(var+eps) as bf16.
    Stats on DVE, rstd on ACT (Ln+Exp keeps one table set), the big
    scale+shift apply on GPSIMD so neither ACT nor DVE serialize it."""
    st6 = pools["small"].tile([tc, 6], F32, name="st6")
    nc.vector.bn_stats(st6[:, :], x_c)
    mv = pools["small"].tile([tc, 2], F32, name="mv")
    nc.vector.bn_aggr(mv[:, :], st6[:, :])
    lnv = pools["small"].tile([tc, 1], F32, name="lnv")
    nc.scalar.activation(lnv[:, :], mv[:, 1:2], AF.Ln, bias=eps[0:tc, 0:1])
    rstd = pools["small"].tile([tc, 1], F32, name="rstd")
    nc.scalar.activation(rstd[:, :], lnv[:, :], AF.Exp, scale=-0.5)
    nb = pools["small"].tile([tc, 1], F32, name="nb")
    nc.vector.tensor_scalar(nb[:, :], mv[:, 0:1], rstd[:, 0:1], -1.0,
                            op0=AL.mult, op1=AL.mult)
    h_c = pools["h"].tile([tc, E], BF16, name="h_c", tag=tag, bufs=5)
    nc.gpsimd.tensor_scalar(h_c[:, :], x_c, rstd[:, 0:1], nb[:, 0:1],
                            op0=AL.mult, op1=AL.add)
    return h_c


def _gen_transpose_pair(nc, pools, h_cs_pair, ident, out):
    """h_cs_pair: list of 2 elems x 2 chunks of [tc,512] bf16 natural ->
    h8 [128, 2, 2, 400] fp8 pair tile (k-pair-interleaved for DoubleRow)
    via PE transposes.  Yields per ec chunk."""
    h8 = pools["t8"].tile([128, 2, 2, 2 * T], FP8, name="h8", bufs=5)
    out["h8"] = h8
    for ec in range(ECH):
        for el in range(2):
            for ci, (t0, tc) in enumerate(TCH):
                ps = pools["ps"].tile([128, tc], BF16, name="t_ps", tag="ps")
                nc.tensor.transpose(
                    ps[:, :], h_cs_pair[el][ci][0:tc, ec * 128:(ec + 1) * 128],
                    ident[0:tc, 0:tc])
                # drains split DVE/ACT to balance the two engines
                if tc == 128:
                    nc.vector.tensor_copy(
                        h8[:, ec // 2, ec % 2,
                           el * T + t0:el * T + t0 + tc], ps[:, :])
                else:
                    nc.scalar.copy(
                        h8[:, ec // 2, ec % 2,
                           el * T + t0:el * T + t0 + tc], ps[:, :])
        yield


def _gen_project_qkT(nc, pools, w_sb, rhs8, name, out, hi,
                     bias_col=None, mask2=None):
    """[128, 400] bf16 pair chunks of (W^T h)^T (carrying the S8 scale).
    bias_col: [128,4] f32 per-channel bias (S8 * LN beta @ W);
    mask2: [128,2T] bf16 per-token mask. Fused into the PSUM drain.
    Yields per oc."""
    for oc in range(4):
        ps = pools["ps"].tile([128, 2 * T], F32, name=f"{name}_ps", tag="ps")
        for g in range(2):
            nc.tensor.matmul(ps[:, :], w_sb[:, g, oc, :, :],
                             rhs8[:, g, :, :], start=(g == 0), stop=(g == 1),
                             perf_mode=DR)
        qk = "q" if name.startswith("q") else "k"
        sb = pools["qkt"].tile([128, 2 * T], BF16, name=f"{name}_sb", tag=qk,
                               bufs=5)
        if bias_col is not None and mask2 is not None:
            nc.vector.scalar_tensor_tensor(sb[:, :], ps[:, :],
                                           bias_col[:, oc:oc + 1], mask2[:, :],
                                           op0=AL.add, op1=AL.mult)
        elif bias_col is not None:
            nc.vector.tensor_scalar(sb[:, :], ps[:, :], bias_col[:, oc:oc + 1],
                                    None, op0=AL.add)
        elif mask2 is not None:
            nc.vector.tensor_mul(sb[:, :], ps[:, :], mask2[:, :])
        else:
            nc.scalar.copy(sb[:, :], ps[:, :])
        hb = pools["qkt"].tile([64, 2 * T], BF16, name=f"{name}_hi", tag="hi",
                               bufs=10)
        nc.sync.dma_start(hb[:, :], sb[64:128, :])
        out.append(sb)
        hi.append(hb)
        yield


def _gen_project_v(nc, pools, wv_sb, h8, off, name, out):
    """v natural [tc, 512] bf16 tiles (carrying S8) for ONE elem.
    fp8 DoubleRow: stationary = h8 pair slices (j-stride 400B), moving =
    host-packed wv [128, 2, 2, 512].  Yields per chunk."""
    for (t0, tc) in TCH:
        ps = pools["ps"].tile([tc, E], F32, name=f"{name}_ps", tag="ps")
        for g in range(2):
            nc.tensor.matmul(ps[:, :],
                             h8[:, g, :, off + t0:off + t0 + tc],
                             wv_sb[:, g, :, :],
                             start=(g == 0), stop=(g == 1), perf_mode=DR)
        sb = pools["v"].tile([tc, E], BF16, name=f"{name}_sb", tag="v", bufs=8)
        nc.scalar.copy(sb[:, :], ps[:, :])
        out.append(sb)
        yield


def _gen_scores_pair(nc, pools, qkt, e0m, e1m):
    """Scores + exp + causal mask for BOTH elems.  Yields per (oc, el)."""
    (qT_lo, qT_hi), (kT_lo, kT_hi) = qkt
    esc = SCALE / (S8 * S8)  # q,k both carry the S8 weight pre-scale
    for oc in range(4):
        for el in range(2):
            off = el * T
            st0 = pools["ps"].tile([128, 2, 200], F32, name="st0", tag="ps")
            st1 = pools["ps"].tile([72, 2, 72], F32, name="st1", tag="ps")
            for hl in range(2):
                qh = (qT_lo, qT_hi)[hl][oc][0:64, off:off + 200]
                kh = (kT_lo, kT_hi)[hl][oc][0:64, off:off + 200]
                nc.tensor.matmul(st0[:, hl, :], kh[:, 0:128], qh)
                nc.tensor.matmul(st1[:, hl, :], kh[:, 128:200],
                                 qh[:, 128:200])
            e0 = pools["e0"].tile([128, 2, 200], BF16, name="e0", bufs=6)
            nc.scalar.activation(e0[:, :, :], st0[:, :, :], AF.Exp, scale=esc)
            e1 = pools["e1"].tile([72, 2, 72], BF16, name="e1", bufs=6)
            nc.scalar.activation(e1[:, :, :], st1[:, :, :], AF.Exp, scale=esc)
            # causal: keep where t - s >= 0 (iota = -p + t), else 0
            e0x = pools["e0"].tile([128, 2, 200], BF16, name="e0x", bufs=16)
            nc.gpsimd.affine_select(
                e0x[:, :, :], e0[:, :, :], pattern=[[0, 2], [1, 200]],
                compare_op=AL.is_ge, fill=0.0, base=0, channel_multiplier=-1)
            e1x = pools["e1"].tile([72, 2, 72], BF16, name="e1x", bufs=16)
            nc.gpsimd.affine_select(
                e1x[:, :, :], e1[:, :, :], pattern=[[0, 2], [1, 72]],
                compare_op=AL.is_ge, fill=0.0, base=0, channel_multiplier=-1)
            e0m[el][oc] = e0x
            e1m[el][oc] = e1x
            yield


def _gen_attn_denoms(nc, pools, e0m, e1m, oneh8, dibs):
    """Softmax denominators d[h, t] = sum_s e -> [8, 200] psum per elem,
    drained to bf16 SBUF.  Emitted EARLY (before the previous ln_stage)
    so the PE->DVE->PE handoff never queues behind LN bn_stats.
    Yields per elem."""
    for el in range(2):
        dps8 = pools["ps"].tile([8, 200], F32, name="dps8", tag="ps")
        for oc in range(4):
            for hl in range(2):
                h = 2 * oc + hl
                nc.tensor.matmul(dps8[:, 0:200], oneh8[:, h, :],
                                 e0m[el][oc][:, hl, :], start=(h == 0),
                                 stop=False, skip_group_check=True)
                nc.tensor.matmul(dps8[:, 128:200], oneh8[0:72, h, :],
                                 e1m[el][oc][:, hl, :], start=False,
                                 stop=(h == 7), skip_group_check=True)
        dib = pools["dinv"].tile([8, 200], BF16, name="dinv8b")
        nc.vector.tensor_copy(dib[:, :], dps8[:, :])
        dibs.append(dib)
        yield


def _gen_attn_av(nc, pools, e0m, e1m, v2, dibs, ohp, o8s):
    """AV on UNNORMALIZED exp tiles; 1/d applied to the AV output via a
    pair-one-hot PE broadcast (reciprocal rides the drain) + one DVE mul
    -> fp8 o8.  The per-channel v bias is folded into the oproj bias
    host-side.  GENERATOR: yields after each (el, oc) unit so dense
    matmul streams can be woven in (keeps the HAM clock-gate released)."""
    for el in range(2):
        for oc in range(4):
            # broadcast d (not 1/d) to the two 64-row head blocks; the
            # reciprocal rides the PSUM->SBUF drain (DVE can only read
            # one PSUM operand per instruction)
            db = pools["ps"].tile([128, 200], F32, name="db_ps", tag="ps")
            nc.tensor.matmul(db[:, :], ohp[:, oc, :], dibs[el][:, :])
            dinv = pools["dinv"].tile([128, 200], F32, name="dinv",
                                      tag="dinvb", bufs=4)
            nc.vector.reciprocal_approx_fast(dinv[:, :], db[:, :])
            ot = pools["ps"].tile([128, 200], F32, name="ot_ps", tag="ps")
            for hl in range(2):
                h = 2 * oc + hl
                hp = hl * 64
                nc.tensor.matmul(ot[hp:hp + 64, 0:200],
                                 v2[el][0][0:128, h * 64:(h + 1) * 64],
                                 e0m[el][oc][:, hl, :], start=True, stop=False,
                                 skip_group_check=True)
                nc.tensor.matmul(ot[hp:hp + 64, 128:200],
                                 v2[el][1][0:72, h * 64:(h + 1) * 64],
                                 e1m[el][oc][:, hl, :], start=False, stop=True,
                                 skip_group_check=True)
            nc.vector.tensor_mul(o8s[el][:, oc // 2, oc % 2, 0:200],
                                 ot[:, :], dinv[:, :])
            yield


def _gen_attn_oproj(nc, pools, o8, wo_sb, bo_row, ones_row, x_cs, skip_bo,
                    new_x):
    """Output projection, fp8 DoubleRow (stationary = o8 pair slices,
    j-stride 208B), + optional rank-1 bias + residual (1/S8^2 unscale
    fused into the drain).  GENERATOR: yields per chunk."""
    for ci, (t0, tc) in enumerate(TCH):
        ps = pools["ps"].tile([tc, E], F32, name="proj_ps", tag="ps")
        if not skip_bo:
            nc.tensor.matmul(ps[:, :], ones_row[0:1, 0:tc], bo_row[0:1, :],
                             start=True, stop=False)
        for g in range(2):
            nc.tensor.matmul(ps[:, :], o8[:, g, :, t0:t0 + tc],
                             wo_sb[:, g, :, :],
                             start=(skip_bo and g == 0), stop=(g == 1),
                             perf_mode=DR)
        xn = pools["res"].tile([tc, E], F32, name="xn", tag="res")
        nc.vector.scalar_tensor_tensor(xn[:, :], ps[:, :], 1.0 / (S8 * S8),
                                       x_cs[ci], op0=AL.mult, op1=AL.add)
        new_x.append(xn)
        yield


def _weave(dense, sparse, nd, ns, dense_head=0, sparse_head=0):
    """Interleave two emission generators (Bresenham by expected step
    counts) so every HAM activity window sees dense PE work -- the
    clock-gate stays at K=8 through the sparse attention phases."""
    for _ in range(dense_head):
        if next(dense, _weave) is _weave:
            dense = None
            break
    for _ in range(sparse_head):
        if sparse is not None and next(sparse, _weave) is _weave:
            sparse = None
            break
    i_d = i_s = 0
    while dense is not None or sparse is not None:
        go_dense = (sparse is None
                    or (dense is not None and i_d * ns <= i_s * nd))
        if go_dense:
            if next(dense, _weave) is _weave:
                dense = None
            else:
                i_d += 1
        else:
            if next(sparse, _weave) is _weave:
                sparse = None
            else:
                i_s += 1


def _gen_empty():
    return iter(())


def _build(bpc, stages=3, skip_bo=False, skip_b2=False, skip_qkb=False):
    nc = bacc.Bacc("TRN2", target_bir_lowering=False, debug=False,
                   enable_asserts=False, num_devices=NCORES)
    dram = {}

    def din(name, shape, dt):
        h = nc.dram_tensor(name, list(shape), dt, kind="ExternalInput")
        dram[name] = h
        return h

    x_d = din("x", (bpc, T, E), F32)
    # host-transposed memory, fp8, k-pair interleaved for DoubleRow
    memt_d = din("memt", (bpc, 2, 2, 128, T), FP8)
    pm_d = din("pm", (bpc, T), BF16)
    sm_d = din("sm", (bpc, T), BF16)
    # weights fp8, pre-scaled by S8.  DoubleRow stationaries (q,k,w1) are
    # host-packed [128, g, oc, j, 128] so each (g,oc) block's two k-tiles
    # are contiguous; moving-side weights (v, o, w2) are host-packed
    # [128, g, j, N].
    wq_sa_d = din("wq_sa", (128, 2, 4, 2, 128), FP8)
    wk_sa_d = din("wk_sa", (128, 2, 4, 2, 128), FP8)
    wv_sa_d = din("wv_sa", (128, 2, 2, E), FP8)
    wo_sa_d = din("wo_sa", (128, 2, 2, E), FP8)
    bo_sa_d = din("bo_sa", (1, E), BF16)
    wq_ca_d = din("wq_ca", (128, 2, 4, 2, 128), FP8)
    wk_ca_d = din("wk_ca", (128, 2, 4, 2, 128), FP8)
    wv_ca_d = din("wv_ca", (128, 2, 2, E), FP8)
    wo_ca_d = din("wo_ca", (128, 2, 2, E), FP8)
    bo_ca_d = din("bo_ca", (1, E), BF16)
    w1_d = din("w1", (128, 2, 16, 2, 128), FP8)
    b1_d = din("b1", (1, F), BF16)
    w2_d = din("w2", (128, 8, 2, E), FP8)
    b2_d = din("b2", (1, E), BF16)
    # per-channel bias columns [128, 4] f32 (S8 * LN beta folded through W)
    bqc_sa_d = din("bqc_sa", (128, 4), F32)
    bkc_sa_d = din("bkc_sa", (128, 4), F32)
    bqc_ca_d = din("bqc_ca", (128, 4), F32)
    out_d = nc.dram_tensor("out", [bpc, T, E], F32, kind="ExternalOutput")

    ones_d = nc.inline_tensor(np.ones((1, E), dtype=NPBF16), name="onesc")
    # [p, h, m] one-hot columns for the [8,200] denominator stack
    oneh8_np = np.zeros((128, 8, 8), dtype=NPBF16)
    for h in range(8):
        oneh8_np[:, h, h] = 1
    oneh8_d = nc.inline_tensor(oneh8_np, name="oneh8c")
    # [h, oc, m] pair-one-hot: broadcast d[2oc] to rows 0:64, d[2oc+1] to
    # rows 64:128
    ohp_np = np.zeros((8, 4, 128), dtype=NPBF16)
    for oc in range(4):
        ohp_np[2 * oc, oc, 0:64] = 1
        ohp_np[2 * oc + 1, oc, 64:128] = 1
    ohp_d = nc.inline_tensor(ohp_np, name="ohpc")
    identb_d = nc.inline_tensor(np.eye(128, dtype=NPBF16), name="identbc")

    with tile.TileContext(nc) as tcx, ExitStack() as ctx:
        pools = {}

        def pool(name, bufs, space="SBUF"):
            pools[name] = ctx.enter_context(
                tcx.tile_pool(name=name, bufs=bufs, space=space))
            return pools[name]

        wpool = pool("w", 1)
        pool("small", 8)
        pool("h", 6)
        pool("t8", 5)
        pool("qkt", 5)
        pool("v", 5)
        pool("e0", 3)
        pool("e1", 3)
        pool("o8", 5)
        pool("dinv", 6)
        pool("res", 24)
        pool("rr", 10)
        pool("mrow", 4)
        pool("mbc", 6)
        pool("ps", 8, space="PSUM")

        def wtile(name, src, shape, rearr=None, dt=BF16, eng=None):
            t = wpool.tile(shape, dt, tag=name, bufs=1, name=name)
            ap = src[:] if rearr is None else src[:].rearrange(rearr, p=128)
            (eng or nc.sync).dma_start(t[...], ap)
            return t

        # SA weights first (sync queue) so pair 0 starts quickly; bulk
        # FFN/CA weights go on the scalar HWDGE queue in parallel
        WC = {}
        eps = wpool.tile([128, 1], F32, tag="eps", bufs=1, name="eps")
        nc.gpsimd.memset(eps[:, :], 1e-5)
        # Weight DMAs are EMITTED from the pipeline section (after pair 0's
        # x/mask loads are queued) so pair 0's LN starts immediately; the
        # functions below close over this dict.
        W = {}

        def load_weights():
            # SA weights on the sync queue (behind pair-0 x); CA/FFN bulk
            # on the scalar HWDGE queue in parallel
            WC["identb"] = wtile("identb", identb_d, [128, 128])
            WC["oneh8"] = wtile("oneh8", oneh8_d, [128, 8, 8])
            WC["ohp"] = wtile("ohp", ohp_d, [8, 4, 128])
            WC["ones"] = wtile("ones", ones_d, [1, E])
            W["wq_sa"] = wtile("wq_sa", wq_sa_d, [128, 2, 4, 2, 128], dt=FP8)
            W["wk_sa"] = wtile("wk_sa", wk_sa_d, [128, 2, 4, 2, 128], dt=FP8)
            W["wv_sa"] = wtile("wv_sa", wv_sa_d, [128, 2, 2, E], dt=FP8)
            W["wo_sa"] = wtile("wo_sa", wo_sa_d, [128, 2, 2, E], dt=FP8)
            W["bqc_sa"] = wtile("bqc_sa", bqc_sa_d, [128, 4], dt=F32)
            W["bkc_sa"] = wtile("bkc_sa", bkc_sa_d, [128, 4], dt=F32)
            W["bo_sa"] = wtile("bo_sa", bo_sa_d, [1, E])
            W["wq_ca"] = wtile("wq_ca", wq_ca_d, [128, 2, 4, 2, 128], dt=FP8,
                               eng=nc.scalar)
            W["wk_ca"] = wtile("wk_ca", wk_ca_d, [128, 2, 4, 2, 128], dt=FP8,
                               eng=nc.scalar)
            W["wv_ca"] = wtile("wv_ca", wv_ca_d, [128, 2, 2, E], dt=FP8,
                               eng=nc.scalar)
            W["wo_ca"] = wtile("wo_ca", wo_ca_d, [128, 2, 2, E], dt=FP8,
                               eng=nc.scalar)
            W["bqc_ca"] = wtile("bqc_ca", bqc_ca_d, [128, 4], dt=F32,
                                eng=nc.scalar)
            W["bo_ca"] = wtile("bo_ca", bo_ca_d, [1, E], eng=nc.scalar)
            W["w1"] = wtile("w1", w1_d, [128, 2, 16, 2, 128], dt=FP8,
                            eng=nc.scalar)
            W["w2"] = wtile("w2", w2_d, [128, 8, 2, E], dt=FP8, eng=nc.scalar)
            W["b2"] = wtile("b2", b2_d, [1, E], eng=nc.scalar)
            # f_b1 (+ folded ln3_b @ w1) in column layout for the relu bias
            b1c = wpool.tile([128, FCH], F32, tag="b1c", bufs=1, name="b1c")
            b1cb = wpool.tile([128, FCH], BF16, tag="b1cb", bufs=1,
                              name="b1cb")
            nc.scalar.dma_start(b1cb[...],
                                b1_d[:].rearrange("o (c p) -> p (o c)",
                                                  p=128))
            nc.vector.tensor_copy(b1c[:, :], b1cb[:, :])
            W["b1c"] = b1c
        # Pre-load the one ACT table set that covers every function we use
        # (Ln, Exp, Relu, Copy) so the auto-insertion pass never needs to
        # switch sets (a switch costs ~1.3us and stalls ACT).
        from concourse.hw_specs import get_activation_tables
        _sets = get_activation_tables(nc.m.arch)
        _aid = list(_sets.keys()).index("natural_log_exp_and_others")
        nc.scalar.add_instruction(mybir.InstLoadActFuncSet(
            name=nc.get_next_instruction_name(), act_func_set_id=_aid,
            ins=[], outs=[]))

        def qkb(name):
            return None if skip_qkb else W[name]

        def load_pair(pr):
            els = (2 * pr, 2 * pr + 1)
            x_el = []
            pm2 = pools["mbc"].tile([128, 2 * T], BF16, name="pm2")
            sm2 = pools["mbc"].tile([128, 2 * T], BF16, name="sm2")
            pmrow2 = pools["mrow"].tile([1, 2 * T], BF16, name="pmrow2",
                                        bufs=3)
            for el, e in enumerate(els):
                x_cs = []
                for ci, (t0, tc) in enumerate(TCH):
                    xt = pools["res"].tile([tc, E], F32, name="x_in",
                                           tag="res")
                    # split x loads across the sync and scalar HWDGE
                    # queues: halves the serial startup DMA and spreads
                    # steady-state queue load
                    eng = nc.sync if (2 * el + ci) % 2 == 0 else nc.scalar
                    eng.dma_start(xt[:, :], x_d[e, t0:t0 + tc, :])
                    x_cs.append(xt)
                x_el.append(x_cs)
                nc.sync.dma_start(pmrow2[0:1, el * T:(el + 1) * T],
                                  pm_d[e:e + 1, :])
                nc.gpsimd.partition_broadcast(pm2[:, el * T:(el + 1) * T],
                                              pmrow2[0:1,
                                                     el * T:(el + 1) * T])
                sm_row = pools["mrow"].tile([1, T], BF16, name="sm_row",
                                            bufs=3)
                nc.sync.dma_start(sm_row[:, :], sm_d[e:e + 1, :])
                nc.gpsimd.partition_broadcast(sm2[:, el * T:(el + 1) * T],
                                              sm_row[:, :])
            return {"els": els, "x_el": x_el, "pm2": pm2, "sm2": sm2}

        def ln_stage(st, key):
            st[key] = [[_layernorm(nc, pools, st["x_el"][el][ci][:, :], tc,
                                   eps, key)
                        for ci, (t0, tc) in enumerate(TCH)]
                       for el in range(2)]

        def gen_front(st, which):
            """Transposes/projections/scores as a generator (~24 steps).
            which='sa' uses h1/pm2 -> v2/e0m; 'ca' uses h2/m8/sm2."""
            out = {}
            if which == "sa":
                yield from _gen_transpose_pair(nc, pools, st["h1"],
                                               WC["identb"],
                                               out)
                h8 = out["h8"]
                qT, kT = [], []
                hi = []
                yield from _gen_project_qkT(nc, pools, W["wq_sa"], h8,
                                            "q_sa", qT, hi,
                                            qkb("bqc_sa"), st["pm2"])
                yield from _gen_project_qkT(nc, pools, W["wk_sa"], h8,
                                            "k_sa", kT, hi,
                                            qkb("bkc_sa"), st["pm2"])
                st["v2"] = [[], []]
                for el in range(2):
                    yield from _gen_project_v(nc, pools, W["wv_sa"], h8,
                                              el * T, "v_sa", st["v2"][el])
                st["e0m"] = [[None] * 4 for _ in range(2)]
                st["e1m"] = [[None] * 4 for _ in range(2)]
                yield from _gen_scores_pair(nc, pools,
                                            ((qT, hi[0:4]), (kT, hi[4:8])),
                                            st["e0m"], st["e1m"])
            else:
                yield from _gen_transpose_pair(nc, pools, st["h2"],
                                               WC["identb"],
                                               out)
                h28 = out["h8"]
                m8 = pools["t8"].tile([128, 2, 2, 2 * T], FP8, name="m8",
                                      bufs=2)
                for el, e in enumerate(st["els"]):
                    nc.scalar.dma_start(
                        m8[:, :, :, el * T:(el + 1) * T],
                        memt_d[e].rearrange("g j p t -> p g j t"))
                qT, kT = [], []
                hi = []
                yield from _gen_project_qkT(nc, pools, W["wq_ca"], h28,
                                            "q_ca", qT, hi,
                                            qkb("bqc_ca"), None)
                yield from _gen_project_qkT(nc, pools, W["wk_ca"], m8,
                                            "k_ca", kT, hi, None,
                                            st["sm2"])
                st["cv2"] = [[], []]
                for el in range(2):
                    yield from _gen_project_v(nc, pools, W["wv_ca"], h28,
                                              el * T, "v_ca",
                                              st["cv2"][el])
                st["ce0m"] = [[None] * 4 for _ in range(2)]
                st["ce1m"] = [[None] * 4 for _ in range(2)]
                yield from _gen_scores_pair(nc, pools,
                                            ((qT, hi[0:4]), (kT, hi[4:8])),
                                            st["ce0m"], st["ce1m"])

        def gen_denoms(st, which):
            key, e0, e1 = (("dibs", "e0m", "e1m") if which == "sa"
                           else ("cdibs", "ce0m", "ce1m"))
            st[key] = []
            yield from _gen_attn_denoms(nc, pools, st[e0], st[e1], WC["oneh8"],
                                        st[key])

        def gen_back(st, which):
            """AV+normalize+oproj as a sparse generator (~12 steps)."""
            if which == "sa":
                e0, e1, v2, dibs = (st["e0m"], st["e1m"], st["v2"],
                                    st["dibs"])
                wo, bo = W["wo_sa"], W["bo_sa"]
            else:
                e0, e1, v2, dibs = (st["ce0m"], st["ce1m"], st["cv2"],
                                    st["cdibs"])
                wo, bo = W["wo_ca"], W["bo_ca"]
            o8s = [pools["o8"].tile([128, 2, 2, 208], FP8, name="o8",
                                    bufs=5) for _ in range(2)]
            yield from _gen_attn_av(nc, pools, e0, e1, v2, dibs,
                                    WC["ohp"], o8s)
            for el in range(2):
                nx = []
                yield from _gen_attn_oproj(nc, pools, o8s[el], wo, bo,
                                           WC["ones"], st["x_el"][el],
                                           skip_bo, nx)
                st["x_el"][el] = nx

        def gen_ffn(st):
            """FFN as a dense generator (~16 steps); LN3 already emitted.
            w1 and w2 both fp8 DoubleRow; relu drains fp8 into fc-pair
            tiles for w2."""
            x_el = st["x_el"]
            out = {}
            yield from _gen_transpose_pair(nc, pools, st["h3"],
                                           WC["identb"], out)
            h38 = out["h8"]
            rr = [pools["rr"].tile([128, 2, 2 * T], FP8, name="rr")
                  for _ in range(FCH // 2)]
            for fc in range(FCH):
                zps = pools["ps"].tile([128, 2 * T], F32, name="z_ps",
                                       tag="ps")
                for g in range(2):
                    nc.tensor.matmul(zps[:, :], W["w1"][:, g, fc, :, :],
                                     h38[:, g, :, :], start=(g == 0),
                                     stop=(g == 1), perf_mode=DR)
                # z carries S8: relu(z/S8 + b1) -> r is O(1) fp8
                nc.scalar.activation(rr[fc // 2][:, fc % 2, :], zps[:, :],
                                     AF.Relu, bias=W["b1c"][:, fc:fc + 1],
                                     scale=1.0 / S8)
                if fc % 2 == 1:
                    yield
            for el, e in enumerate(st["els"]):
                for ci, (t0, tc) in enumerate(TCH):
                    yps = pools["ps"].tile([tc, E], F32, name="y_ps",
                                           tag="ps")
                    if not skip_b2:
                        nc.tensor.matmul(yps[:, :], WC["ones"][0:1, 0:tc],
                                         W["b2"][0:1, :], start=True,
                                         stop=False)
                    for fcp in range(FCH // 2):
                        nc.tensor.matmul(
                            yps[:, :],
                            rr[fcp][:, :, el * T + t0:el * T + t0 + tc],
                            W["w2"][:, fcp, :, :],
                            start=(skip_b2 and fcp == 0),
                            stop=(fcp == 7), perf_mode=DR)
                    yout = pools["res"].tile([tc, E], F32, name="yout",
                                             tag="res")
                    nc.vector.scalar_tensor_tensor(yout[:, :], yps[:, :],
                                                   1.0 / S8,
                                                   x_el[el][ci][:, :],
                                                   op0=AL.mult, op1=AL.add)
                    nc.sync.dma_start(out_d[e, t0:t0 + tc, :], yout[:, :])
                    yield

        def drain(g):
            for _ in g:
                pass

        def chain(*gens):
            for g in gens:
                yield from g

        # 2-deep software pipeline over pairs.  LN for each stage is
        # emitted as early as its input allows (ln1 a full stage early)
        # so engine queues drain the LN chain long before the PE's
        # transposes need it; in between, every cross-engine latency has
        # an independent block of PE work queued ahead of it.
        # 2-deep software pipeline over pairs with DENSE/SPARSE weaving:
        # the attention phases (scores/denoms/AV -- short matmuls) are
        # interleaved at ~1us granularity with the dense DR matmul
        # streams (FFN, next pair's projections) so the HAM activity
        # window always sees a busy PE and keeps the clock at 2.4 GHz.
        npairs = bpc // 2
        sts = [load_pair(0)]
        ln_stage(sts[0], "h1")
        load_weights()
        drain(gen_front(sts[0], "sa"))
        drain(gen_denoms(sts[0], "sa"))
        for pr in range(npairs):
            st = sts[pr]
            if pr + 1 < npairs:
                stn = load_pair(pr + 1)
                sts.append(stn)
            sparse = gen_back(st, "sa")
            if pr >= 1:
                _weave(gen_ffn(sts[pr - 1]), sparse, 16, 12, sparse_head=6)
            else:
                drain(sparse)
            ln_stage(st, "h2")
            if pr + 1 < npairs:
                ln_stage(sts[pr + 1], "h1")
            drain(gen_front(st, "ca"))
            sparse2 = chain(gen_denoms(st, "ca"), gen_back(st, "ca"))
            if pr + 1 < npairs:
                _weave(gen_front(sts[pr + 1], "sa"), sparse2, 24, 14,
                       dense_head=5)
                drain(gen_denoms(sts[pr + 1], "sa"))
            else:
                drain(sparse2)
            ln_stage(st, "h3")
        drain(gen_ffn(sts[npairs - 1]))

    nc.compile()
    return nc


def _host_prep(inputs, bpc, core):
    """Build the in_map for one core."""
    s = slice(core * bpc, (core + 1) * bpc)

    def rearr(w, g=None):  # (H, E, D) -> [E, H*D] fp32, optionally row-scaled
        m = np.transpose(np.asarray(w, np.float32), (1, 0, 2)).reshape(E, E)
        if g is not None:
            m = m * np.asarray(g, np.float32)[:, None]
        return m

    def w8(m):  # [K, N] f32 -> [128, K/256, 2, N] fp8, pre-scaled by S8
        K, N = m.shape
        r = (np.asarray(m, np.float32) * S8).reshape(K // 256, 2, 128, N)
        r = r.transpose(2, 0, 1, 3)
        return np.ascontiguousarray(np.clip(r, -240, 240)).astype(NPFP8)

    def w8s(m):  # stationary pack: [K, N] -> [128, K/256, N/128, 2, 128]
        K, N = m.shape
        r = (np.asarray(m, np.float32) * S8).reshape(K // 256, 2, 128,
                                                     N // 128, 128)
        r = r.transpose(2, 0, 3, 1, 4)
        return np.ascontiguousarray(np.clip(r, -240, 240)).astype(NPFP8)

    def b16(a):
        return np.ascontiguousarray(np.asarray(a, np.float32)).astype(NPBF16)

    def f32c(a):
        return np.ascontiguousarray(np.asarray(a, np.float32))

    def bcol(b):  # [E] row bias -> [128, 4] per-channel columns, x S8
        return np.ascontiguousarray(
            (np.asarray(b, np.float32) * S8).reshape(4, 128).T)

    g1 = np.asarray(inputs["ln1_g"], np.float32)
    b1n = np.asarray(inputs["ln1_b"], np.float32)
    g2 = np.asarray(inputs["ln2_g"], np.float32)
    b2n = np.asarray(inputs["ln2_b"], np.float32)
    g3 = np.asarray(inputs["ln3_g"], np.float32)
    b3n = np.asarray(inputs["ln3_b"], np.float32)

    def wr(w):  # raw rearranged fp32 (for beta @ W rows)
        return np.transpose(np.asarray(w, np.float32), (1, 0, 2)).reshape(E, E)

    mem = np.asarray(inputs["memory"], np.float32)[s]         # [bpc, T, E]
    memt = np.transpose(mem, (0, 2, 1)).reshape(bpc, 2, 2, 128, T)

    # v bias (LN beta @ Wv) folded through Wo into the oproj bias; the
    # oproj drain unscales by 1/S8^2 so bias rows carry S8^2
    bo_sa = (np.asarray(inputs["sa_bo"], np.float32)
             + (b1n @ wr(inputs["sa_wv"])) @ np.asarray(inputs["sa_wo"],
                                                        np.float32))
    bo_ca = (np.asarray(inputs["ca_bo"], np.float32)
             + (b2n @ wr(inputs["ca_wv"])) @ np.asarray(inputs["ca_wo"],
                                                        np.float32))

    return {
        "x": f32c(inputs["idx"][s]),
        "memt": np.ascontiguousarray(np.clip(memt, -240, 240)).astype(NPFP8),
        "pm": b16(inputs["pred_mask"][s] != 0),
        "sm": b16(inputs["src_mask"][s] != 0),
        "wq_sa": w8s(rearr(inputs["sa_wq"], g1)),
        "wk_sa": w8s(rearr(inputs["sa_wk"], g1)),
        "wv_sa": w8(rearr(inputs["sa_wv"], g1)),
        "wo_sa": w8(np.asarray(inputs["sa_wo"], np.float32)),
        "bo_sa": b16(bo_sa * S8 * S8).reshape(1, E),
        "bqc_sa": bcol(b1n @ wr(inputs["sa_wq"])),
        "bkc_sa": bcol(b1n @ wr(inputs["sa_wk"])),
        "wq_ca": w8s(rearr(inputs["ca_wq"], g2)),
        "wk_ca": w8s(rearr(inputs["ca_wk"])),
        "wv_ca": w8(rearr(inputs["ca_wv"], g2)),
        "wo_ca": w8(np.asarray(inputs["ca_wo"], np.float32)),
        "bo_ca": b16(bo_ca * S8 * S8).reshape(1, E),
        "bqc_ca": bcol(b2n @ wr(inputs["ca_wq"])),
        "w1": w8s(np.asarray(inputs["f_w1"], np.float32) * g3[:, None]),
        "b1": b16(np.asarray(inputs["f_b1"], np.float32)
                  + b3n @ np.asarray(inputs["f_w1"], np.float32)).reshape(1, F),
        "w2": w8(np.asarray(inputs["f_w2"], np.float32)),
        "b2": b16(np.asarray(inputs["f_b2"], np.float32) * S8).reshape(1, E),
    }


def get_program(bpc, skip_bo=True, skip_b2=True, skip_qkb=True):
    key = (bpc, skip_bo, skip_b2, skip_qkb)
    if key not in _programs:
        _programs[key] = _build(bpc, skip_bo=skip_bo, skip_b2=skip_b2,
                                skip_qkb=skip_qkb)
    return _programs[key]


def kernel(**inputs) -> np.ndarray:
    bpc = B // NCORES
    # specialize: drop bias work that is identically zero
    b1n = np.asarray(inputs["ln1_b"], np.float32)
    b2n = np.asarray(inputs["ln2_b"], np.float32)
    skip_qkb = bool(not b1n.any() and not b2n.any())
    skip_bo = bool(
        not np.any(np.asarray(inputs["sa_bo"], np.float32))
        and not np.any(np.asarray(inputs["ca_bo"], np.float32))
        and skip_qkb)
    skip_b2 = bool(not np.any(np.asarray(inputs["f_b2"], np.float32)))
    nc = get_program(bpc, skip_bo, skip_b2, skip_qkb)
    in_maps = [_host_prep(inputs, bpc, c) for c in range(NCORES)]
    res = run_bass_kernel_spmd(nc, in_maps, core_ids=list(range(NCORES)))
    out = np.concatenate([res.results[c]["out"] for c in range(NCORES)], axis=0)
    return out.astype(np.float32)
